# revision 51
# baseline (speedup 1.0000x reference)
"""2-layer 4-head GAT (DGL GATConv-style) as a distributed Bass/Tile kernel
on 8 Trainium2 NeuronCores.

Sharding: destination nodes split 6272/core (49 tiles of 128). Per layer every
core redundantly computes the dense projections for ALL nodes into a bf16
"dcat" table in its HBM: row n = [X@W (256) | X@w_el (4) | pad] at a 768B
stride (w_el is host-prefolded W@al). The edge phase then, per 128-destination
tile, dma_gathers the [fs|el] rows by src (one gather per table half; the
random-row HBM latency of these gathers is the kernel's floor), computes
er per edge with a matmul against a host-precomputed transposed one-hot
indicator (indT[d,e] = [dst_local(e)==d], streamed from HBM), forms
p = exp(leakyrelu(el_src + er_dst)), scales messages by p in place, and
scatter-adds into PSUM [128, 260] with indicator matmuls (the untransposed
one-hot, also streamed). er/res for a core's own 6272 nodes come from a small
per-core dense pass over per-core-sliced inputs (no gather). The epilogue
normalizes by the attention sums, means heads, adds the residual, and applies
LayerNorm+ReLU; layer-0 results are AllGathered, and the layer-1 epilogue also
applies the prediction head.

The dcat table has 2 halves of 25088 rows so dma_gather's int16 indices reach
every row; pad slots gather row 0 (harmless: their one-hot columns are zero).
"""
import contextlib
import ctypes
import os
import sys
import types

import numpy as np

sys.path.insert(0, "/opt/trn_rl_repo")

import ml_dtypes  # noqa: E402

# ---------------------------------------------------------------------------
# Shim 1: antenv.axon_hooks (missing in this image) so trace=True works.
# ---------------------------------------------------------------------------
_ntff_hook = None


def _install_axon_hooks_shim():
    global _ntff_hook
    if "antenv.axon_hooks" in sys.modules:
        return
    try:
        import antenv
    except ImportError:
        return
    mod = types.ModuleType("antenv.axon_hooks")

    def set_axon_ntff_profile_hook(h):
        global _ntff_hook
        _ntff_hook = h

    def get_axon_ntff_profile_hook():
        return _ntff_hook

    mod.set_axon_ntff_profile_hook = set_axon_ntff_profile_hook
    mod.get_axon_ntff_profile_hook = get_axon_ntff_profile_hook
    sys.modules["antenv.axon_hooks"] = mod
    antenv.axon_hooks = mod

    so_path = "/opt/axon/libaxon_pjrt.so"
    try:
        lib = ctypes.CDLL(so_path)
    except OSError:
        return
    if not hasattr(lib, "axon_start_nrt_profile"):
        return
    lib.axon_start_nrt_profile.argtypes = [
        ctypes.POINTER(ctypes.c_int64),
        ctypes.c_size_t,
    ]
    lib.axon_start_nrt_profile.restype = ctypes.c_int64
    lib.axon_stop_nrt_profile.argtypes = [ctypes.c_char_p]
    lib.axon_stop_nrt_profile.restype = ctypes.c_int64

    @contextlib.contextmanager
    def _hook(output_dir, device_ids):
        import jax

        jax.devices()
        if device_ids:
            ids = (ctypes.c_int64 * len(device_ids))(*device_ids)
            rc = lib.axon_start_nrt_profile(ids, len(device_ids))
        else:
            rc = lib.axon_start_nrt_profile(None, 0)
        if rc != 0:
            raise RuntimeError(f"axon_start_nrt_profile rc={rc}")
        try:
            yield
        finally:
            n = lib.axon_stop_nrt_profile(str(output_dir).encode())
            if n < 0:
                raise RuntimeError(f"axon_stop_nrt_profile rc={n}")
            print(f"profile: {n} file(s) written to {output_dir}", file=sys.stderr)

    set_axon_ntff_profile_hook(_hook)


_install_axon_hooks_shim()

import concourse.bass as bass  # noqa: E402
import concourse.bacc as bacc  # noqa: E402
import concourse.mybir as mybir  # noqa: E402
import concourse.tile as tile  # noqa: E402
from concourse.bass_utils import run_bass_kernel_spmd  # noqa: E402


# ---------------------------------------------------------------------------
# Problem constants (kernel.py is self-contained; shapes are hardcoded).
# ---------------------------------------------------------------------------
N, E = 50000, 800000
IN, HID, H, OUT = 128, 64, 4, 64
NEG_SLOPE = 0.2
EPS = 1e-5

P = 128
NCORES = 8
T = 49                       # dst node tiles per core
NPC = T * P                  # 6272 nodes per core
N_PAD = NCORES * NPC         # 50176 (node space)
NHALF = N_PAD // 2           # 25088 dcat rows per half
RL = 384                     # dcat row length (bf16) -> 768B stride
C_EL = 256
DCOLS = 264                  # occupied dcat cols (fs 256 | el 4 | junk 4)
WCOLS = 328                  # wcat cols: [W 256 | el 4 | er 4 | res 64]
TINY = 1e-30
TILES_G = N_PAD // P         # 392 global tiles

F32 = mybir.dt.float32
BF16 = mybir.dt.bfloat16
I16 = mybir.dt.int16
OP = mybir.AluOpType
AF = mybir.ActivationFunctionType
AX = mybir.AxisListType


def _cdiv(a, b):
    return (a + b - 1) // b


# ---------------------------------------------------------------------------
# Host-side edge preprocessing
# ---------------------------------------------------------------------------
def _wrap_idx(flat):
    """dma_gather index layout: idx j -> [j%16, j//16], replicated to 128
    partitions."""
    n = len(flat)
    assert n % 128 == 0
    cols = n // 16
    w = np.zeros((16, cols), np.int16)
    w[np.arange(n) % 16, np.arange(n) // 16] = flat
    return np.tile(w, (8, 1))


def _prep_edges(src, dst):
    src = np.asarray(src).astype(np.int64)
    dst = np.asarray(dst).astype(np.int64)
    order = np.argsort(dst, kind="stable")
    src = src[order]
    dst = dst[order]

    bounds = np.searchsorted(dst, np.arange(0, N_PAD + 1, P))

    lo_lists = [[None] * T for _ in range(NCORES)]
    hi_lists = [[None] * T for _ in range(NCORES)]
    for c in range(NCORES):
        for t in range(T):
            gt = c * T + t
            e0, e1 = bounds[gt], bounds[gt + 1]
            s = np.asarray(src[e0:e1])
            d = np.asarray(dst[e0:e1]) - gt * P
            is_lo = s < NHALF
            lo_lists[c][t] = (s[is_lo], d[is_lo])
            hi_lists[c][t] = (s[~is_lo] - NHALF, d[~is_lo])

    NLO = [
        max(1, max(len(lo_lists[c][t][0]) for c in range(NCORES)))
        for t in range(T)
    ]
    NHI = [
        max(1, max(len(hi_lists[c][t][0]) for c in range(NCORES)))
        for t in range(T)
    ]
    K_lo = [_cdiv(NLO[t], P) for t in range(T)]
    K_hi = [_cdiv(NHI[t], P) for t in range(T)]
    KT = [K_lo[t] + K_hi[t] for t in range(T)]

    # idx16 column layout per tile: [lo: 8*K_lo | hi: 8*K_hi]
    cols_per_tile = [8 * KT[t] for t in range(T)]
    idx_off = np.cumsum([0] + cols_per_tile).tolist()
    ind_off = np.cumsum([0] + KT).tolist()  # chunk offset per tile
    IDX_COLS = int(idx_off[-1])
    SUM_KT = int(ind_off[-1])

    idx16 = np.zeros((NCORES, 128, IDX_COLS), np.int16)
    ind_np = np.zeros((NCORES, 128, SUM_KT * 128), ml_dtypes.bfloat16)
    indT_np = np.zeros((NCORES, 128, SUM_KT * 128), ml_dtypes.bfloat16)

    def perm(s):
        # dcat row permutation (see dense_block): 4-tile groups of 512 rows,
        # row = (gt//4)*512 + p*4 + gt%4 for node (gt, p)
        return (s // 512) * 512 + (s % 128) * 4 + (s // 128) % 4

    for c in range(NCORES):
        for t in range(T):
            co = idx_off[t]
            s_lo, d_lo = lo_lists[c][t]
            s_hi, d_hi = hi_lists[c][t]
            # --- src gathers
            pad_lo = np.zeros(K_lo[t] * P, np.int64)
            pad_lo[: len(s_lo)] = perm(s_lo)
            idx16[c, :, co : co + 8 * K_lo[t]] = _wrap_idx(pad_lo)
            co += 8 * K_lo[t]
            pad_hi = np.zeros(K_hi[t] * P, np.int64)
            pad_hi[: len(s_hi)] = perm(s_hi)
            idx16[c, :, co : co + 8 * K_hi[t]] = _wrap_idx(pad_hi)
            # --- one-hot indicators: slot j (chunk j//128, lane j%128)
            nslots = KT[t] * P
            dl = np.full(nslots, -1, np.int64)
            dl[: len(d_lo)] = d_lo
            off = K_lo[t] * P
            dl[off : off + len(d_hi)] = d_hi
            j = np.arange(nslots)
            valid = dl >= 0
            jv, dv = j[valid], dl[valid]
            base = ind_off[t] * 128
            # ind[e_lane, chunk*128 + d] = 1
            ind_np[c, jv % 128, base + (jv // 128) * 128 + dv] = 1.0
            # indT[d_lane, chunk*128 + e] = 1
            indT_np[c, dv, base + (jv // 128) * 128 + (jv % 128)] = 1.0

    return dict(
        K_lo=K_lo,
        K_hi=K_hi,
        NLO=NLO,
        NHI=NHI,
        KT=KT,
        KMAX=int(max(KT)),
        idx_off=[int(x) for x in idx_off],
        ind_off=[int(x) for x in ind_off],
        IDX_COLS=IDX_COLS,
        SUM_KT=SUM_KT,
        idx16=idx16,
        ind_np=ind_np,
        indT_np=indT_np,
    )


# ---------------------------------------------------------------------------
# Bass program
# ---------------------------------------------------------------------------
PIECES = [(0, 12), (12, 24), (24, 36), (36, 45), (45, 47), (47, 49)]


def _build_program(ep):
    K_lo, K_hi, KT = ep["K_lo"], ep["K_hi"], ep["KT"]
    NLO, NHI = ep["NLO"], ep["NHI"]
    idx_off, ind_off = ep["idx_off"], ep["ind_off"]
    IDX_COLS, SUM_KT, KMAX = ep["IDX_COLS"], ep["SUM_KT"], ep["KMAX"]

    nc = bacc.Bacc("TRN2", target_bir_lowering=False, debug=False,
                   num_devices=NCORES)

    featsT_in = nc.dram_tensor("featsT", [IN, N_PAD], BF16, kind="ExternalInput")
    fownT_in = nc.dram_tensor("featsT_own", [IN, NPC], BF16,
                              kind="ExternalInput")
    wcat0_in = nc.dram_tensor("wcat0", [IN, WCOLS], BF16, kind="ExternalInput")
    wcat1_in = nc.dram_tensor("wcat1", [HID, WCOLS], BF16, kind="ExternalInput")
    predw_in = nc.dram_tensor("predw", [HID, OUT], F32, kind="ExternalInput")
    aux_in = nc.dram_tensor("aux", [P, 8 * 64], F32, kind="ExternalInput")
    ident_in = nc.dram_tensor("ident", [P, P], F32, kind="ExternalInput")
    idx_in = nc.dram_tensor("idx16", [P, IDX_COLS], I16, kind="ExternalInput")
    ind_in = nc.dram_tensor("indhot", [P, SUM_KT * 128], BF16,
                            kind="ExternalInput")
    indT_in = nc.dram_tensor("indhotT", [P, SUM_KT * 128], BF16,
                             kind="ExternalInput")
    out_t = nc.dram_tensor("out", [NPC, OUT], F32, kind="ExternalOutput")

    with tile.TileContext(nc) as tc:
        with (
            tc.tile_pool(name="const", bufs=1) as constp,
            tc.tile_pool(name="persist", bufs=1) as persist,
            tc.tile_pool(name="dense", bufs=3) as densep,
            tc.tile_pool(name="edge", bufs=3) as edgep,
            tc.tile_pool(name="indp", bufs=2) as indp,
            tc.tile_pool(name="epi", bufs=2) as epip,
            tc.tile_pool(name="tps", bufs=2, space="PSUM") as tps,
            tc.tile_pool(name="dps", bufs=2, space="PSUM") as dps,
            tc.tile_pool(name="aps", bufs=2, space="PSUM") as aps,
            tc.tile_pool(name="eps", bufs=2, space="PSUM") as epsp,
            tc.tile_pool(name="dram", bufs=1, space="DRAM") as dram,
        ):
            # ---- constants / persistent data
            wcat0 = constp.tile([IN, WCOLS], BF16)
            nc.sync.dma_start(out=wcat0[:], in_=wcat0_in[:, :])
            wcat1 = constp.tile([HID, WCOLS], BF16)
            nc.sync.dma_start(out=wcat1[:], in_=wcat1_in[:, :])
            predw = constp.tile([HID, OUT], F32)
            nc.sync.dma_start(out=predw[:], in_=predw_in[:, :])
            aux = constp.tile([P, 8 * 64], F32)
            nc.sync.dma_start(out=aux[:], in_=aux_in[:, :])
            ident = constp.tile([P, P], F32)
            nc.sync.dma_start(out=ident[:], in_=ident_in[:, :])
            idx16 = persist.tile([P, IDX_COLS], I16)
            nc.sync.dma_start(out=idx16[:], in_=idx_in[:, :])

            gml = [aux[:, 0:64], aux[:, 128:192]]
            bml = [aux[:, 64:128], aux[:, 192:256]]
            resbl = [aux[:, 256:320], aux[:, 320:384]]
            predb = aux[:, 384:448]
            eps_col = aux[:, 448:449]

            dcat = [
                dram.tile([N_PAD, RL], BF16, name="dcat0", tag="dcat0"),
                dram.tile([N_PAD, RL], BF16, name="dcat1", tag="dcat1"),
            ]
            h1ownT_p = [
                dram.tile([HID, (p1 - p0) * P], BF16, name=f"h1ownT{j}",
                          tag=f"h1ownT{j}")
                for j, (p0, p1) in enumerate(PIECES)
            ]
            h1fullT_p = [
                dram.tile([NCORES * HID, (p1 - p0) * P], BF16,
                          name=f"h1fullT{j}", tag=f"h1fullT{j}",
                          addr_space="Shared")
                for j, (p0, p1) in enumerate(PIECES)
            ]

            def piece_of(t):
                for j, (p0, p1) in enumerate(PIECES):
                    if p0 <= t < p1:
                        return j, p0
                raise AssertionError(t)

            conv = persist.tile([P, T, 260], F32)
            res_own_l = [
                persist.tile([P, T, HID], F32, tag=f"res_own{i}",
                             name=f"res_own{i}")
                for i in range(2)
            ]
            er_own_l = [
                persist.tile([P, T * 4], BF16, tag=f"er_own{i}",
                             name=f"er_own{i}")
                for i in range(2)
            ]

            # =============== phases ===============
            def mini_dense_tile(li, t, din, wcat):
                """er/res for one own-node tile (per-core transposed input)."""
                if li == 0:
                    src = fownT_in[0:din, t * P : (t + 1) * P]
                else:
                    j, p0 = piece_of(t)
                    src = h1ownT_p[j][0:din, (t - p0) * P : (t - p0 + 1) * P]
                xT = densep.tile([din, P], BF16, tag="mxT")
                nc.sync.dma_start(out=xT[:], in_=src)
                er_ps = dps.tile([P, 68], F32, tag="dc_ps")
                nc.tensor.matmul(
                    out=er_ps[:], lhsT=xT[:], rhs=wcat[:, 260:WCOLS],
                    start=True, stop=True,
                )
                nc.scalar.copy(
                    out=er_own_l[li][:, t * 4 : (t + 1) * 4],
                    in_=er_ps[:, 0:4],
                )
                nc.scalar.copy(out=res_own_l[li][:, t, :], in_=er_ps[:, 4:68])

            def dense_block(li, wcat, gt0, nb, src_ap, dve_cast=False):
                dc = dcat[li]
                din = IN if li == 0 else HID
                xT = densep.tile([din, nb * P], BF16, tag="xT")
                nc.sync.dma_start(out=xT[:], in_=src_ap)
                dcb = densep.tile([P, nb, RL], BF16, tag="dcb")
                for j in range(nb):
                    dc_ps = dps.tile([P, DCOLS], F32, tag="dc_ps")
                    nc.tensor.matmul(
                        out=dc_ps[:],
                        lhsT=xT[:, j * P : (j + 1) * P],
                        rhs=wcat[:, 0:DCOLS],
                        start=True,
                        stop=True,
                    )
                    if dve_cast and j % 2 == 1:
                        nc.vector.tensor_copy(out=dcb[:, j, 0:DCOLS], in_=dc_ps[:])
                    else:
                        nc.scalar.copy(out=dcb[:, j, 0:DCOLS], in_=dc_ps[:])
                # permuted dcat layout: node (gt,p) -> row (gt//4)*512+p*4+gt%4
                # so each perm-group write is 4 contiguous rows per partition
                i = 0
                while i < nb:
                    gt = gt0 + i
                    G, j0 = gt // 4, gt % 4
                    seg = min(4 - j0, nb - i)
                    nc.sync.dma_start(
                        out=dc[G * 512 : (G + 1) * 512, :].rearrange(
                            "(p j) c -> p j c", j=4
                        )[:, j0 : j0 + seg, :],
                        in_=dcb[:, i : i + seg, :],
                    )
                    i += seg

            def dense_phase0(wcat):
                for gt0 in range(0, TILES_G, 8):
                    nb = min(8, TILES_G - gt0)
                    dense_block(
                        0, wcat, gt0, nb,
                        featsT_in[0:IN, gt0 * P : (gt0 + nb) * P],
                        dve_cast=True,
                    )

            def l1_dense_blocks():
                """(piece, gt0, nb, src_ap) for all layer-1 dense blocks."""
                out = []
                for j, (p0, p1) in enumerate(PIECES):
                    for band in range(NCORES):
                        for t0 in range(p0, p1, 4):
                            nb = min(4, p1 - t0)
                            out.append((
                                j, band * T + t0, nb,
                                h1fullT_p[j][
                                    band * HID : (band + 1) * HID,
                                    (t0 - p0) * P : (t0 - p0 + nb) * P,
                                ],
                            ))
                return out

            def edge_tile(li, t):
                dc = dcat[li]
                if True:
                    kl, kh, kt = K_lo[t], K_hi[t], KT[t]
                    # exact-ish index counts (rounded to 16): slots beyond
                    # them are never gathered; every consumer below excludes
                    # those stale slots, so they are never read.
                    nlo = min(kl * P, (NLO[t] + 15) // 16 * 16)
                    nhi = min(kh * P, (NHI[t] + 15) // 16 * 16)
                    pl = nlo - (kl - 1) * P  # valid partitions, last lo chunk
                    ph = nhi - (kh - 1) * P  # valid partitions, last hi chunk
                    io = idx_off[t]
                    g = edgep.tile([P, kt, RL], BF16, tag="gbuf")
                    nc.gpsimd.dma_gather(
                        out_ap=g[:, 0:kl, :],
                        in_ap=dc[0:NHALF, :],
                        idxs_ap=idx16[:, io : io + 8 * kl],
                        num_idxs=nlo,
                        num_idxs_reg=nlo,
                        elem_size=RL,
                        elem_step=RL,
                        single_packet=(nlo <= 1024),
                    )
                    io2 = io + 8 * kl
                    nc.gpsimd.dma_gather(
                        out_ap=g[:, kl:kt, :],
                        in_ap=dc[NHALF:N_PAD, :],
                        idxs_ap=idx16[:, io2 : io2 + 8 * kh],
                        num_idxs=nhi,
                        num_idxs_reg=nhi,
                        elem_size=RL,
                        elem_step=RL,
                        single_packet=(nhi <= 1024),
                    )
                    # gathered regions: (partitions, chunk-range) pairs
                    regions = []
                    if pl == P:
                        regions.append((P, 0, kl))
                    else:
                        if kl > 1:
                            regions.append((P, 0, kl - 1))
                        regions.append((pl, kl - 1, kl))
                    if ph == P:
                        regions.append((P, kl, kt))
                    else:
                        if kh > 1:
                            regions.append((P, kl, kt - 1))
                        regions.append((ph, kt - 1, kt))
                    # streamed one-hot indicators for this tile
                    c0 = ind_off[t] * 128
                    c1 = ind_off[t + 1] * 128
                    indsb = indp.tile([P, kt * 128], BF16, tag="indsb")
                    nc.sync.dma_start(out=indsb[:], in_=ind_in[:, c0:c1])
                    indTsb = indp.tile([P, kt * 128], BF16, tag="indTsb")
                    nc.sync.dma_start(out=indTsb[:], in_=indT_in[:, c0:c1])
                    # er per edge: indT.T @ er_tile, per chunk
                    er_ps = epsp.tile([P, KMAX * 4], F32, tag="er_ps")
                    for k in range(kt):
                        nc.tensor.matmul(
                            out=er_ps[:, k * 4 : (k + 1) * 4],
                            lhsT=indTsb[:, k * 128 : (k + 1) * 128],
                            rhs=er_own_l[li][:, t * 4 : (t + 1) * 4],
                            start=True,
                            stop=True,
                        )
                    # x = el_src + er_dst; p = exp(leakyrelu(x))
                    el = g[:, :, C_EL : C_EL + 4]
                    x = edgep.tile([P, kt, 4], F32, tag="xlog")
                    nc.vector.tensor_tensor(
                        out=x[:],
                        in0=el,
                        in1=er_ps[:, 0 : kt * 4].rearrange(
                            "p (k f) -> p k f", f=4
                        ),
                        op=OP.add,
                    )
                    x2 = edgep.tile([P, kt, 4], F32, tag="xlog2")
                    nc.vector.tensor_scalar_mul(x2[:], x[:], NEG_SLOPE)
                    nc.vector.tensor_tensor(
                        out=x2[:], in0=x2[:], in1=x[:], op=OP.max
                    )
                    for (np_, k0, k1) in regions:
                        nc.scalar.activation(
                            out=g[0:np_, k0:k1, C_EL : C_EL + 4],
                            in_=x2[0:np_, k0:k1, :],
                            func=AF.Exp,
                        )
                    # msg *= p (broadcast over the 64 cols of each head)
                    for (np_, k0, k1) in regions:
                        msg4 = g[0:np_, k0:k1, 0:256].rearrange(
                            "p k (h f) -> p k h f", f=64
                        )
                        nc.vector.tensor_tensor(
                            out=msg4,
                            in0=msg4,
                            in1=g[0:np_, k0:k1, C_EL : C_EL + 4].to_broadcast(
                                [np_, k1 - k0, 4, 64]
                            ),
                            op=OP.mult,
                        )
                    # indicator matmuls: PSUM[n, :] += sum_e (dst(e)==n)*row_e
                    ps = aps.tile([P, 260], F32, tag="agg")
                    for k in range(kt):
                        np_ = P
                        if k == kl - 1:
                            np_ = pl
                        elif k == kt - 1:
                            np_ = ph
                        nc.tensor.matmul(
                            out=ps[:],
                            lhsT=indsb[0:np_, k * 128 : (k + 1) * 128],
                            rhs=g[0:np_, k, 0:260],
                            start=(k == 0),
                            stop=(k == kt - 1),
                        )
                    nc.scalar.copy(out=conv[:, t, :], in_=ps[:])

            def epilogue_tile(li, t):
                g_ln, b_ln, resb = gml[li], bml[li], resbl[li]
                S = conv[:, t, 0:256]
                asum = conv[:, t, 256:260]
                r4 = epip.tile([P, 4], F32, tag="r4")
                nc.vector.tensor_scalar(
                    r4[:], asum, 4.0, TINY, OP.mult, OP.max
                )
                rec = epip.tile([P, 4], F32, tag="rec")
                nc.vector.reciprocal(rec[:], r4[:])
                m = epip.tile([P, HID], F32, tag="m")
                nc.vector.tensor_scalar(
                    m[:], S[:, 0:64], rec[:, 0:1], None, OP.mult
                )
                for h in range(1, H):
                    nc.vector.scalar_tensor_tensor(
                        out=m[:],
                        in0=S[:, 64 * h : 64 * (h + 1)],
                        scalar=rec[:, h : h + 1],
                        in1=m[:],
                        op0=OP.mult,
                        op1=OP.add,
                    )
                xr = epip.tile([P, HID], F32, tag="xr")
                nc.vector.tensor_tensor(
                    out=xr[:], in0=m[:], in1=res_own_l[li][:, t, :],
                    op=OP.add
                )
                nc.vector.tensor_tensor(
                    out=xr[:], in0=xr[:], in1=resb, op=OP.add
                )
                # LayerNorm + ReLU
                stat = epip.tile([P, 8], F32, tag="stat")
                nc.vector.tensor_reduce(
                    out=stat[:, 0:1], in_=xr[:], axis=AX.X, op=OP.add
                )
                nc.vector.tensor_scalar_mul(
                    stat[:, 1:2], stat[:, 0:1], 1.0 / HID
                )
                xc = epip.tile([P, HID], F32, tag="xc")
                nc.vector.tensor_scalar(
                    xc[:], xr[:], stat[:, 1:2], None, OP.subtract
                )
                sq = epip.tile([P, HID], F32, tag="sq")
                nc.scalar.activation(
                    out=sq[:], in_=xc[:], func=AF.Square,
                    accum_out=stat[:, 2:3],
                )
                nc.scalar.activation(
                    out=stat[:, 3:4], in_=stat[:, 2:3], func=AF.Sqrt,
                    bias=eps_col, scale=1.0 / HID,
                )
                nc.vector.reciprocal(stat[:, 4:5], stat[:, 3:4])
                y = epip.tile([P, HID], F32, tag="y")
                nc.vector.scalar_tensor_tensor(
                    out=y[:], in0=xc[:], scalar=stat[:, 4:5], in1=g_ln,
                    op0=OP.mult, op1=OP.mult,
                )
                ht = epip.tile([P, HID], F32, tag="ht")
                nc.vector.tensor_tensor(out=ht[:], in0=y[:], in1=b_ln, op=OP.add)
                nc.vector.tensor_scalar_max(ht[:], ht[:], 0.0)
                hT_ps = tps.tile([HID, P], F32, tag="xT_ps")
                nc.tensor.transpose(out=hT_ps[:], in_=ht[:], identity=ident[:])
                if li == 0:
                    j, p0 = piece_of(t)
                    hTb = epip.tile([HID, P], BF16, tag="hTb")
                    nc.scalar.copy(out=hTb[:], in_=hT_ps[:])
                    nc.sync.dma_start(
                        out=h1ownT_p[j][:, (t - p0) * P : (t - p0 + 1) * P],
                        in_=hTb[:],
                    )
                else:
                    hT = epip.tile([HID, P], F32, tag="hT")
                    nc.scalar.copy(out=hT[:], in_=hT_ps[:])
                    hd_ps = dps.tile([P, OUT], F32, tag="dc_ps")
                    nc.tensor.matmul(
                        out=hd_ps[:], lhsT=hT[:], rhs=predw[:],
                        start=True, stop=True,
                    )
                    ob = epip.tile([P, OUT], F32, tag="ob")
                    nc.vector.tensor_tensor(
                        out=ob[:], in0=hd_ps[:], in1=predb, op=OP.add
                    )
                    nc.sync.dma_start(
                        out=out_t[t * P : (t + 1) * P, :], in_=ob[:]
                    )

            def collective_piece(j):
                nc.gpsimd.collective_compute(
                    "AllGather",
                    OP.bypass,
                    replica_groups=[list(range(NCORES))],
                    ins=[h1ownT_p[j][:].opt()],
                    outs=[h1fullT_p[j][:].opt()],
                )

            piece_ends = {p1: j for j, (p0, p1) in enumerate(PIECES)}

            def layer(li, interleave=None):
                """Edge + pipelined epilogue (tile t-2) + piece collectives
                + optionally interleaved next-layer work."""

                def epi(t):
                    epilogue_tile(li, t)
                    if li == 0 and (t + 1) in piece_ends:
                        collective_piece(piece_ends[t + 1])

                for t in range(T):
                    edge_tile(li, t)
                    if t >= 2:
                        epi(t - 2)
                    if interleave:
                        for emit in interleave.get(t, ()):
                            emit()
                for t in (T - 2, T - 1):
                    epi(t)
                if interleave:
                    for emit in interleave.get(-1, ()):
                        emit()

            # =============== schedule ===============
            dense_phase0(wcat0)
            for t in range(T):
                mini_dense_tile(0, t, IN, wcat0)

            # interleave schedule for layer-1 prep inside the layer-0 loop:
            # mini-dense tile k once epilogue(k) is done (~k+4); dense blocks
            # of piece j a few tiles after piece j's collective fires.
            sched = {}

            def at(t, fn):
                sched.setdefault(t, []).append(fn)

            for k in range(T):
                at(k + 4 if k + 4 < T else -1,
                   (lambda k=k: mini_dense_tile(1, k, HID, wcat1)))
            start_of = {0: 18, 1: 30, 2: 42, 3: -1, 4: -1, 5: -1}
            counts = {}
            leftovers = []
            for (j, gt0, nb, src_ap) in l1_dense_blocks():
                s = start_of[j]
                slot = -1
                if s >= 0:
                    k = counts.get(j, 0)
                    counts[j] = k + 1
                    slot = s + k // 2
                if slot < 0 or slot >= T:
                    leftovers.append((gt0 // T >= 4, j, gt0, nb, src_ap))
                    continue
                at(slot, (lambda w=wcat1, g=gt0, n=nb, a=src_ap:
                          dense_block(1, w, g, n, a)))
            # post-loop leftovers: half-0 bands first so layer-1 lo-gathers
            # unblock as early as possible
            for (_, j, gt0, nb, src_ap) in sorted(
                leftovers, key=lambda x: (x[0], x[1])
            ):
                at(-1, (lambda w=wcat1, g=gt0, n=nb, a=src_ap:
                        dense_block(1, w, g, n, a)))

            layer(0, interleave=sched)
            layer(1)

    nc.compile()
    return nc


# ---------------------------------------------------------------------------
# Host entry point
# ---------------------------------------------------------------------------
def kernel(feats, src, dst, W0, al0, ar0, resw0, resb0, g0, b0,
           W1, al1, ar1, resw1, resb1, g1, b1, predw, predb):
    f32 = np.float32
    feats = np.asarray(feats, f32)
    W0 = np.asarray(W0, f32)
    al0 = np.asarray(al0, f32)
    ar0 = np.asarray(ar0, f32)
    resw0 = np.asarray(resw0, f32)
    W1 = np.asarray(W1, f32)
    al1 = np.asarray(al1, f32)
    ar1 = np.asarray(ar1, f32)
    resw1 = np.asarray(resw1, f32)
    predw_np = np.asarray(predw, f32)

    ep = _prep_edges(src, dst)
    nc = _build_program(ep)

    feats_pad = np.zeros((N_PAD, IN), f32)
    feats_pad[:N] = feats

    def fold(W, a):
        return (W.reshape(W.shape[0], H, HID) * a[None]).sum(-1)

    # wcat: [W (256) | el (4) | er (4) | res (64)]
    wcat0 = np.concatenate([W0, fold(W0, al0), fold(W0, ar0), resw0], axis=1)
    wcat1 = np.concatenate([W1, fold(W1, al1), fold(W1, ar1), resw1], axis=1)

    aux = np.zeros((P, 8 * 64), f32)
    aux[:, 0:64] = np.asarray(g0, f32)[None]
    aux[:, 64:128] = np.asarray(b0, f32)[None]
    aux[:, 128:192] = np.asarray(g1, f32)[None]
    aux[:, 192:256] = np.asarray(b1, f32)[None]
    aux[:, 256:320] = np.asarray(resb0, f32)[None]
    aux[:, 320:384] = np.asarray(resb1, f32)[None]
    aux[:, 384:448] = np.asarray(predb, f32)[None]
    aux[:, 448] = EPS

    ident = np.eye(P, dtype=f32)

    featsT = np.ascontiguousarray(feats_pad.T.astype(ml_dtypes.bfloat16))

    shared = {
        "featsT": featsT,
        "wcat0": np.ascontiguousarray(wcat0.astype(ml_dtypes.bfloat16)),
        "wcat1": np.ascontiguousarray(wcat1.astype(ml_dtypes.bfloat16)),
        "predw": predw_np,
        "aux": aux,
        "ident": ident,
    }
    in_maps = [
        {
            **shared,
            "featsT_own": np.ascontiguousarray(
                featsT[:, c * NPC : (c + 1) * NPC]
            ),
            "idx16": ep["idx16"][c],
            "indhot": ep["ind_np"][c],
            "indhotT": ep["indT_np"][c],
        }
        for c in range(NCORES)
    ]

    trace = os.environ.get("GAT_TRACE", "0") == "1"
    res = run_bass_kernel_spmd(
        nc, in_maps, core_ids=list(range(NCORES)), trace=trace
    )
    if trace and res.exec_time_ns is not None:
        print(f"HW exec time: {res.exec_time_ns} ns")
        if res.instructions_and_trace is not None:
            print(f"trace: {res.instructions_and_trace[1]}")

    out = np.concatenate([res.results[c]["out"] for c in range(NCORES)], axis=0)
    return np.ascontiguousarray(out[:N]).astype(np.float32)


# revision 55
# speedup vs baseline: 1.0309x; 1.0309x over previous
"""2-layer 4-head GAT (DGL GATConv-style) as a distributed Bass/Tile kernel
on 8 Trainium2 NeuronCores.

Sharding: destination nodes split 6272/core (49 tiles of 128). Per layer every
core redundantly computes the dense projections for ALL nodes into a bf16
"dcat" table in its HBM: row n = [X@W (256) | X@w_el (4) | pad] at a 768B
stride (w_el is host-prefolded W@al). The edge phase then, per 128-destination
tile, dma_gathers the [fs|el] rows by src (one gather per table half; the
random-row HBM latency of these gathers is the kernel's floor), computes
er per edge with a matmul against a host-precomputed transposed one-hot
indicator (indT[d,e] = [dst_local(e)==d], streamed from HBM), forms
p = exp(leakyrelu(el_src + er_dst)), scales messages by p in place, and
scatter-adds into PSUM [128, 260] with indicator matmuls (the untransposed
one-hot, also streamed). er/res for a core's own 6272 nodes come from a small
per-core dense pass over per-core-sliced inputs (no gather). The epilogue
normalizes by the attention sums, means heads, adds the residual, and applies
LayerNorm+ReLU; layer-0 results are AllGathered, and the layer-1 epilogue also
applies the prediction head.

The dcat table has 2 halves of 25088 rows so dma_gather's int16 indices reach
every row; pad slots gather row 0 (harmless: their one-hot columns are zero).
"""
import contextlib
import ctypes
import os
import sys
import types

import numpy as np

sys.path.insert(0, "/opt/trn_rl_repo")

import ml_dtypes  # noqa: E402

# ---------------------------------------------------------------------------
# Shim 1: antenv.axon_hooks (missing in this image) so trace=True works.
# ---------------------------------------------------------------------------
_ntff_hook = None


def _install_axon_hooks_shim():
    global _ntff_hook
    if "antenv.axon_hooks" in sys.modules:
        return
    try:
        import antenv
    except ImportError:
        return
    mod = types.ModuleType("antenv.axon_hooks")

    def set_axon_ntff_profile_hook(h):
        global _ntff_hook
        _ntff_hook = h

    def get_axon_ntff_profile_hook():
        return _ntff_hook

    mod.set_axon_ntff_profile_hook = set_axon_ntff_profile_hook
    mod.get_axon_ntff_profile_hook = get_axon_ntff_profile_hook
    sys.modules["antenv.axon_hooks"] = mod
    antenv.axon_hooks = mod

    so_path = "/opt/axon/libaxon_pjrt.so"
    try:
        lib = ctypes.CDLL(so_path)
    except OSError:
        return
    if not hasattr(lib, "axon_start_nrt_profile"):
        return
    lib.axon_start_nrt_profile.argtypes = [
        ctypes.POINTER(ctypes.c_int64),
        ctypes.c_size_t,
    ]
    lib.axon_start_nrt_profile.restype = ctypes.c_int64
    lib.axon_stop_nrt_profile.argtypes = [ctypes.c_char_p]
    lib.axon_stop_nrt_profile.restype = ctypes.c_int64

    @contextlib.contextmanager
    def _hook(output_dir, device_ids):
        import jax

        jax.devices()
        if device_ids:
            ids = (ctypes.c_int64 * len(device_ids))(*device_ids)
            rc = lib.axon_start_nrt_profile(ids, len(device_ids))
        else:
            rc = lib.axon_start_nrt_profile(None, 0)
        if rc != 0:
            raise RuntimeError(f"axon_start_nrt_profile rc={rc}")
        try:
            yield
        finally:
            n = lib.axon_stop_nrt_profile(str(output_dir).encode())
            if n < 0:
                raise RuntimeError(f"axon_stop_nrt_profile rc={n}")
            print(f"profile: {n} file(s) written to {output_dir}", file=sys.stderr)

    set_axon_ntff_profile_hook(_hook)


_install_axon_hooks_shim()

import concourse.bass as bass  # noqa: E402
import concourse.bacc as bacc  # noqa: E402
import concourse.mybir as mybir  # noqa: E402
import concourse.tile as tile  # noqa: E402
from concourse.bass_utils import run_bass_kernel_spmd  # noqa: E402


# ---------------------------------------------------------------------------
# Problem constants (kernel.py is self-contained; shapes are hardcoded).
# ---------------------------------------------------------------------------
N, E = 50000, 800000
IN, HID, H, OUT = 128, 64, 4, 64
NEG_SLOPE = 0.2
EPS = 1e-5

P = 128
NCORES = 8
T = 49                       # dst node tiles per core
NPC = T * P                  # 6272 nodes per core
N_PAD = NCORES * NPC         # 50176 (node space)
NHALF = N_PAD // 2           # 25088 dcat rows per half
RL = 384                     # dcat row length (bf16) -> 768B stride
C_EL = 256
DCOLS = 264                  # occupied dcat cols (fs 256 | el 4 | junk 4)
WCOLS = 328                  # wcat cols: [W 256 | el 4 | er 4 | res 64]
TINY = 1e-30
TILES_G = N_PAD // P         # 392 global tiles

F32 = mybir.dt.float32
BF16 = mybir.dt.bfloat16
I16 = mybir.dt.int16
OP = mybir.AluOpType
AF = mybir.ActivationFunctionType
AX = mybir.AxisListType


def _cdiv(a, b):
    return (a + b - 1) // b


# ---------------------------------------------------------------------------
# Host-side edge preprocessing
# ---------------------------------------------------------------------------
def _wrap_idx(flat):
    """dma_gather index layout: idx j -> [j%16, j//16], replicated to 128
    partitions."""
    n = len(flat)
    assert n % 128 == 0
    cols = n // 16
    w = np.zeros((16, cols), np.int16)
    w[np.arange(n) % 16, np.arange(n) // 16] = flat
    return np.tile(w, (8, 1))


def _prep_edges(src, dst):
    src = np.asarray(src).astype(np.int64)
    dst = np.asarray(dst).astype(np.int64)
    order = np.argsort(dst, kind="stable")
    src = src[order]
    dst = dst[order]

    bounds = np.searchsorted(dst, np.arange(0, N_PAD + 1, P))

    lo_lists = [[None] * T for _ in range(NCORES)]
    hi_lists = [[None] * T for _ in range(NCORES)]
    for c in range(NCORES):
        for t in range(T):
            gt = c * T + t
            e0, e1 = bounds[gt], bounds[gt + 1]
            s = np.asarray(src[e0:e1])
            d = np.asarray(dst[e0:e1]) - gt * P
            is_lo = s < NHALF
            lo_lists[c][t] = (s[is_lo], d[is_lo])
            hi_lists[c][t] = (s[~is_lo] - NHALF, d[~is_lo])

    NLO = [
        max(1, max(len(lo_lists[c][t][0]) for c in range(NCORES)))
        for t in range(T)
    ]
    NHI = [
        max(1, max(len(hi_lists[c][t][0]) for c in range(NCORES)))
        for t in range(T)
    ]
    K_lo = [_cdiv(NLO[t], P) for t in range(T)]
    K_hi = [_cdiv(NHI[t], P) for t in range(T)]
    KT = [K_lo[t] + K_hi[t] for t in range(T)]

    # idx16 column layout per tile: [lo: 8*K_lo | hi: 8*K_hi]
    cols_per_tile = [8 * KT[t] for t in range(T)]
    idx_off = np.cumsum([0] + cols_per_tile).tolist()
    ind_off = np.cumsum([0] + KT).tolist()  # chunk offset per tile
    IDX_COLS = int(idx_off[-1])
    SUM_KT = int(ind_off[-1])

    idx16 = np.zeros((NCORES, 128, IDX_COLS), np.int16)
    ind_np = np.zeros((NCORES, 128, SUM_KT * 128), ml_dtypes.bfloat16)
    indT_np = np.zeros((NCORES, 128, SUM_KT * 128), ml_dtypes.bfloat16)

    def perm(s):
        # dcat row permutation (see dense_block): 4-tile groups of 512 rows,
        # row = (gt//4)*512 + p*4 + gt%4 for node (gt, p)
        return (s // 512) * 512 + (s % 128) * 4 + (s // 128) % 4

    for c in range(NCORES):
        for t in range(T):
            co = idx_off[t]
            s_lo, d_lo = lo_lists[c][t]
            s_hi, d_hi = hi_lists[c][t]
            # --- src gathers
            pad_lo = np.zeros(K_lo[t] * P, np.int64)
            pad_lo[: len(s_lo)] = perm(s_lo)
            idx16[c, :, co : co + 8 * K_lo[t]] = _wrap_idx(pad_lo)
            co += 8 * K_lo[t]
            pad_hi = np.zeros(K_hi[t] * P, np.int64)
            pad_hi[: len(s_hi)] = perm(s_hi)
            idx16[c, :, co : co + 8 * K_hi[t]] = _wrap_idx(pad_hi)
            # --- one-hot indicators: slot j (chunk j//128, lane j%128)
            nslots = KT[t] * P
            dl = np.full(nslots, -1, np.int64)
            dl[: len(d_lo)] = d_lo
            off = K_lo[t] * P
            dl[off : off + len(d_hi)] = d_hi
            j = np.arange(nslots)
            valid = dl >= 0
            jv, dv = j[valid], dl[valid]
            base = ind_off[t] * 128
            # ind[e_lane, chunk*128 + d] = 1
            ind_np[c, jv % 128, base + (jv // 128) * 128 + dv] = 1.0
            # indT[d_lane, chunk*128 + e] = 1
            indT_np[c, dv, base + (jv // 128) * 128 + (jv % 128)] = 1.0

    return dict(
        K_lo=K_lo,
        K_hi=K_hi,
        NLO=NLO,
        NHI=NHI,
        KT=KT,
        KMAX=int(max(KT)),
        idx_off=[int(x) for x in idx_off],
        ind_off=[int(x) for x in ind_off],
        IDX_COLS=IDX_COLS,
        SUM_KT=SUM_KT,
        idx16=idx16,
        ind_np=ind_np,
        indT_np=indT_np,
    )


# ---------------------------------------------------------------------------
# Bass program
# ---------------------------------------------------------------------------
PIECES = [(0, 12), (12, 24), (24, 36), (36, 45), (45, 49)]


def _build_program(ep):
    K_lo, K_hi, KT = ep["K_lo"], ep["K_hi"], ep["KT"]
    NLO, NHI = ep["NLO"], ep["NHI"]
    idx_off, ind_off = ep["idx_off"], ep["ind_off"]
    IDX_COLS, SUM_KT, KMAX = ep["IDX_COLS"], ep["SUM_KT"], ep["KMAX"]

    nc = bacc.Bacc("TRN2", target_bir_lowering=False, debug=False,
                   num_devices=NCORES)

    featsT_in = nc.dram_tensor("featsT", [IN, N_PAD], BF16, kind="ExternalInput")
    fownT_in = nc.dram_tensor("featsT_own", [IN, NPC], BF16,
                              kind="ExternalInput")
    wcat0_in = nc.dram_tensor("wcat0", [IN, WCOLS], BF16, kind="ExternalInput")
    wcat1_in = nc.dram_tensor("wcat1", [HID, WCOLS], BF16, kind="ExternalInput")
    predw_in = nc.dram_tensor("predw", [HID, OUT], F32, kind="ExternalInput")
    aux_in = nc.dram_tensor("aux", [P, 8 * 64], F32, kind="ExternalInput")
    ident_in = nc.dram_tensor("ident", [P, P], F32, kind="ExternalInput")
    idx_in = nc.dram_tensor("idx16", [P, IDX_COLS], I16, kind="ExternalInput")
    ind_in = nc.dram_tensor("indhot", [P, SUM_KT * 128], BF16,
                            kind="ExternalInput")
    indT_in = nc.dram_tensor("indhotT", [P, SUM_KT * 128], BF16,
                             kind="ExternalInput")
    out_t = nc.dram_tensor("out", [NPC, OUT], F32, kind="ExternalOutput")

    with tile.TileContext(nc) as tc:
        with (
            tc.tile_pool(name="const", bufs=1) as constp,
            tc.tile_pool(name="persist", bufs=1) as persist,
            tc.tile_pool(name="dense", bufs=3) as densep,
            tc.tile_pool(name="edge", bufs=3) as edgep,
            tc.tile_pool(name="indp", bufs=2) as indp,
            tc.tile_pool(name="epi", bufs=2) as epip,
            tc.tile_pool(name="tps", bufs=2, space="PSUM") as tps,
            tc.tile_pool(name="dps", bufs=2, space="PSUM") as dps,
            tc.tile_pool(name="aps", bufs=2, space="PSUM") as aps,
            tc.tile_pool(name="eps", bufs=2, space="PSUM") as epsp,
            tc.tile_pool(name="dram", bufs=1, space="DRAM") as dram,
        ):
            # ---- constants / persistent data
            wcat0 = constp.tile([IN, WCOLS], BF16)
            nc.sync.dma_start(out=wcat0[:], in_=wcat0_in[:, :])
            wcat1 = constp.tile([HID, WCOLS], BF16)
            nc.sync.dma_start(out=wcat1[:], in_=wcat1_in[:, :])
            predw = constp.tile([HID, OUT], F32)
            nc.sync.dma_start(out=predw[:], in_=predw_in[:, :])
            aux = constp.tile([P, 8 * 64], F32)
            nc.sync.dma_start(out=aux[:], in_=aux_in[:, :])
            ident = constp.tile([P, P], F32)
            nc.sync.dma_start(out=ident[:], in_=ident_in[:, :])
            idx16 = persist.tile([P, IDX_COLS], I16)
            nc.sync.dma_start(out=idx16[:], in_=idx_in[:, :])

            gml = [aux[:, 0:64], aux[:, 128:192]]
            bml = [aux[:, 64:128], aux[:, 192:256]]
            resbl = [aux[:, 256:320], aux[:, 320:384]]
            predb = aux[:, 384:448]
            eps_col = aux[:, 448:449]

            dcat = [
                dram.tile([N_PAD, RL], BF16, name="dcat0", tag="dcat0"),
                dram.tile([N_PAD, RL], BF16, name="dcat1", tag="dcat1"),
            ]
            h1ownT_p = [
                dram.tile([HID, (p1 - p0) * P], BF16, name=f"h1ownT{j}",
                          tag=f"h1ownT{j}")
                for j, (p0, p1) in enumerate(PIECES)
            ]
            h1fullT_p = [
                dram.tile([NCORES * HID, (p1 - p0) * P], BF16,
                          name=f"h1fullT{j}", tag=f"h1fullT{j}",
                          addr_space="Shared")
                for j, (p0, p1) in enumerate(PIECES)
            ]

            def piece_of(t):
                for j, (p0, p1) in enumerate(PIECES):
                    if p0 <= t < p1:
                        return j, p0
                raise AssertionError(t)

            conv = persist.tile([P, T, 260], F32)
            res_own_l = [
                persist.tile([P, T, HID], F32, tag=f"res_own{i}",
                             name=f"res_own{i}")
                for i in range(2)
            ]
            er_own_l = [
                persist.tile([P, T * 4], BF16, tag=f"er_own{i}",
                             name=f"er_own{i}")
                for i in range(2)
            ]

            # =============== phases ===============
            def mini_dense_tile(li, t, din, wcat):
                """er/res for one own-node tile (per-core transposed input)."""
                if li == 0:
                    src = fownT_in[0:din, t * P : (t + 1) * P]
                else:
                    j, p0 = piece_of(t)
                    src = h1ownT_p[j][0:din, (t - p0) * P : (t - p0 + 1) * P]
                xT = densep.tile([din, P], BF16, tag="mxT")
                nc.sync.dma_start(out=xT[:], in_=src)
                er_ps = dps.tile([P, 68], F32, tag="dc_ps")
                nc.tensor.matmul(
                    out=er_ps[:], lhsT=xT[:], rhs=wcat[:, 260:WCOLS],
                    start=True, stop=True,
                )
                nc.scalar.copy(
                    out=er_own_l[li][:, t * 4 : (t + 1) * 4],
                    in_=er_ps[:, 0:4],
                )
                nc.scalar.copy(out=res_own_l[li][:, t, :], in_=er_ps[:, 4:68])

            def dense_block(li, wcat, gt0, nb, src_ap, dve_cast=False):
                dc = dcat[li]
                din = IN if li == 0 else HID
                xT = densep.tile([din, nb * P], BF16, tag="xT")
                nc.sync.dma_start(out=xT[:], in_=src_ap)
                dcb = densep.tile([P, nb, RL], BF16, tag="dcb")
                for j in range(nb):
                    dc_ps = dps.tile([P, DCOLS], F32, tag="dc_ps")
                    nc.tensor.matmul(
                        out=dc_ps[:],
                        lhsT=xT[:, j * P : (j + 1) * P],
                        rhs=wcat[:, 0:DCOLS],
                        start=True,
                        stop=True,
                    )
                    if dve_cast and j % 2 == 1:
                        nc.vector.tensor_copy(out=dcb[:, j, 0:DCOLS], in_=dc_ps[:])
                    else:
                        nc.scalar.copy(out=dcb[:, j, 0:DCOLS], in_=dc_ps[:])
                # permuted dcat layout: node (gt,p) -> row (gt//4)*512+p*4+gt%4
                # so each perm-group write is 4 contiguous rows per partition
                i = 0
                while i < nb:
                    gt = gt0 + i
                    G, j0 = gt // 4, gt % 4
                    seg = min(4 - j0, nb - i)
                    nc.sync.dma_start(
                        out=dc[G * 512 : (G + 1) * 512, :].rearrange(
                            "(p j) c -> p j c", j=4
                        )[:, j0 : j0 + seg, :],
                        in_=dcb[:, i : i + seg, :],
                    )
                    i += seg

            def dense_phase0(wcat):
                for gt0 in range(0, TILES_G, 8):
                    nb = min(8, TILES_G - gt0)
                    dense_block(
                        0, wcat, gt0, nb,
                        featsT_in[0:IN, gt0 * P : (gt0 + nb) * P],
                        dve_cast=True,
                    )

            def l1_dense_blocks():
                """(piece, gt0, nb, src_ap) for all layer-1 dense blocks."""
                out = []
                for j, (p0, p1) in enumerate(PIECES):
                    for band in range(NCORES):
                        for t0 in range(p0, p1, 4):
                            nb = min(4, p1 - t0)
                            out.append((
                                j, band * T + t0, nb,
                                h1fullT_p[j][
                                    band * HID : (band + 1) * HID,
                                    (t0 - p0) * P : (t0 - p0 + nb) * P,
                                ],
                            ))
                return out

            def edge_tile(li, t):
                dc = dcat[li]
                if True:
                    kl, kh, kt = K_lo[t], K_hi[t], KT[t]
                    io = idx_off[t]
                    g = edgep.tile([P, kt, RL], BF16, tag="gbuf")
                    nc.gpsimd.dma_gather(
                        out_ap=g[:, 0:kl, :],
                        in_ap=dc[0:NHALF, :],
                        idxs_ap=idx16[:, io : io + 8 * kl],
                        num_idxs=kl * P,
                        num_idxs_reg=kl * P,
                        elem_size=RL,
                        elem_step=RL,
                        single_packet=(kl * P <= 1024),
                    )
                    io2 = io + 8 * kl
                    nc.gpsimd.dma_gather(
                        out_ap=g[:, kl:kt, :],
                        in_ap=dc[NHALF:N_PAD, :],
                        idxs_ap=idx16[:, io2 : io2 + 8 * kh],
                        num_idxs=kh * P,
                        num_idxs_reg=kh * P,
                        elem_size=RL,
                        elem_step=RL,
                        single_packet=(kh * P <= 1024),
                    )
                    # streamed one-hot indicators for this tile
                    c0 = ind_off[t] * 128
                    c1 = ind_off[t + 1] * 128
                    indsb = indp.tile([P, kt * 128], BF16, tag="indsb")
                    nc.sync.dma_start(out=indsb[:], in_=ind_in[:, c0:c1])
                    indTsb = indp.tile([P, kt * 128], BF16, tag="indTsb")
                    nc.sync.dma_start(out=indTsb[:], in_=indT_in[:, c0:c1])
                    # er per edge: indT.T @ er_tile, per chunk
                    er_ps = epsp.tile([P, KMAX * 4], F32, tag="er_ps")
                    for k in range(kt):
                        nc.tensor.matmul(
                            out=er_ps[:, k * 4 : (k + 1) * 4],
                            lhsT=indTsb[:, k * 128 : (k + 1) * 128],
                            rhs=er_own_l[li][:, t * 4 : (t + 1) * 4],
                            start=True,
                            stop=True,
                        )
                    # x = el_src + er_dst; p = exp(leakyrelu(x))
                    el = g[:, :, C_EL : C_EL + 4]
                    x = edgep.tile([P, kt, 4], F32, tag="xlog")
                    nc.vector.tensor_tensor(
                        out=x[:],
                        in0=el,
                        in1=er_ps[:, 0 : kt * 4].rearrange(
                            "p (k f) -> p k f", f=4
                        ),
                        op=OP.add,
                    )
                    x2 = edgep.tile([P, kt, 4], F32, tag="xlog2")
                    nc.vector.tensor_scalar_mul(x2[:], x[:], NEG_SLOPE)
                    nc.vector.tensor_tensor(
                        out=x2[:], in0=x2[:], in1=x[:], op=OP.max
                    )
                    nc.scalar.activation(out=el, in_=x2[:], func=AF.Exp)
                    # msg *= p (broadcast over the 64 cols of each head)
                    msg4 = g[:, :, 0:256].rearrange("p k (h f) -> p k h f", f=64)
                    nc.vector.tensor_tensor(
                        out=msg4,
                        in0=msg4,
                        in1=el.to_broadcast([P, kt, 4, 64]),
                        op=OP.mult,
                    )
                    # indicator matmuls: PSUM[n, :] += sum_e (dst(e)==n)*row_e
                    ps = aps.tile([P, 260], F32, tag="agg")
                    for k in range(kt):
                        nc.tensor.matmul(
                            out=ps[:],
                            lhsT=indsb[:, k * 128 : (k + 1) * 128],
                            rhs=g[:, k, 0:260],
                            start=(k == 0),
                            stop=(k == kt - 1),
                        )
                    nc.scalar.copy(out=conv[:, t, :], in_=ps[:])

            def epilogue_tile(li, t):
                g_ln, b_ln, resb = gml[li], bml[li], resbl[li]
                S = conv[:, t, 0:256]
                asum = conv[:, t, 256:260]
                r4 = epip.tile([P, 4], F32, tag="r4")
                nc.vector.tensor_scalar(
                    r4[:], asum, 4.0, TINY, OP.mult, OP.max
                )
                rec = epip.tile([P, 4], F32, tag="rec")
                nc.vector.reciprocal(rec[:], r4[:])
                m = epip.tile([P, HID], F32, tag="m")
                nc.vector.tensor_scalar(
                    m[:], S[:, 0:64], rec[:, 0:1], None, OP.mult
                )
                for h in range(1, H):
                    nc.vector.scalar_tensor_tensor(
                        out=m[:],
                        in0=S[:, 64 * h : 64 * (h + 1)],
                        scalar=rec[:, h : h + 1],
                        in1=m[:],
                        op0=OP.mult,
                        op1=OP.add,
                    )
                xr = epip.tile([P, HID], F32, tag="xr")
                nc.vector.tensor_tensor(
                    out=xr[:], in0=m[:], in1=res_own_l[li][:, t, :],
                    op=OP.add
                )
                nc.vector.tensor_tensor(
                    out=xr[:], in0=xr[:], in1=resb, op=OP.add
                )
                # LayerNorm + ReLU
                stat = epip.tile([P, 8], F32, tag="stat")
                nc.vector.tensor_reduce(
                    out=stat[:, 0:1], in_=xr[:], axis=AX.X, op=OP.add
                )
                nc.vector.tensor_scalar_mul(
                    stat[:, 1:2], stat[:, 0:1], 1.0 / HID
                )
                xc = epip.tile([P, HID], F32, tag="xc")
                nc.vector.tensor_scalar(
                    xc[:], xr[:], stat[:, 1:2], None, OP.subtract
                )
                sq = epip.tile([P, HID], F32, tag="sq")
                nc.scalar.activation(
                    out=sq[:], in_=xc[:], func=AF.Square,
                    accum_out=stat[:, 2:3],
                )
                nc.scalar.activation(
                    out=stat[:, 3:4], in_=stat[:, 2:3], func=AF.Sqrt,
                    bias=eps_col, scale=1.0 / HID,
                )
                nc.vector.reciprocal(stat[:, 4:5], stat[:, 3:4])
                y = epip.tile([P, HID], F32, tag="y")
                nc.vector.scalar_tensor_tensor(
                    out=y[:], in0=xc[:], scalar=stat[:, 4:5], in1=g_ln,
                    op0=OP.mult, op1=OP.mult,
                )
                ht = epip.tile([P, HID], F32, tag="ht")
                nc.vector.tensor_tensor(out=ht[:], in0=y[:], in1=b_ln, op=OP.add)
                nc.vector.tensor_scalar_max(ht[:], ht[:], 0.0)
                hT_ps = tps.tile([HID, P], F32, tag="xT_ps")
                nc.tensor.transpose(out=hT_ps[:], in_=ht[:], identity=ident[:])
                if li == 0:
                    j, p0 = piece_of(t)
                    hTb = epip.tile([HID, P], BF16, tag="hTb")
                    nc.scalar.copy(out=hTb[:], in_=hT_ps[:])
                    nc.sync.dma_start(
                        out=h1ownT_p[j][:, (t - p0) * P : (t - p0 + 1) * P],
                        in_=hTb[:],
                    )
                else:
                    hT = epip.tile([HID, P], F32, tag="hT")
                    nc.scalar.copy(out=hT[:], in_=hT_ps[:])
                    hd_ps = dps.tile([P, OUT], F32, tag="dc_ps")
                    nc.tensor.matmul(
                        out=hd_ps[:], lhsT=hT[:], rhs=predw[:],
                        start=True, stop=True,
                    )
                    ob = epip.tile([P, OUT], F32, tag="ob")
                    nc.vector.tensor_tensor(
                        out=ob[:], in0=hd_ps[:], in1=predb, op=OP.add
                    )
                    nc.sync.dma_start(
                        out=out_t[t * P : (t + 1) * P, :], in_=ob[:]
                    )

            def collective_piece(j):
                nc.gpsimd.collective_compute(
                    "AllGather",
                    OP.bypass,
                    replica_groups=[list(range(NCORES))],
                    ins=[h1ownT_p[j][:].opt()],
                    outs=[h1fullT_p[j][:].opt()],
                )

            piece_ends = {p1: j for j, (p0, p1) in enumerate(PIECES)}

            def layer(li, interleave=None):
                """Edge + pipelined epilogue (tile t-2) + piece collectives
                + optionally interleaved next-layer work."""

                def epi(t):
                    epilogue_tile(li, t)
                    if li == 0 and (t + 1) in piece_ends:
                        collective_piece(piece_ends[t + 1])

                for t in range(T):
                    edge_tile(li, t)
                    if t >= 2:
                        epi(t - 2)
                    if interleave:
                        for emit in interleave.get(t, ()):
                            emit()
                for t in (T - 2, T - 1):
                    epi(t)
                if interleave:
                    for emit in interleave.get(-1, ()):
                        emit()

            # =============== schedule ===============
            dense_phase0(wcat0)
            for t in range(T):
                mini_dense_tile(0, t, IN, wcat0)

            # interleave schedule for layer-1 prep inside the layer-0 loop:
            # mini-dense tile k once epilogue(k) is done (~k+4); dense blocks
            # of piece j a few tiles after piece j's collective fires.
            sched = {}

            def at(t, fn):
                sched.setdefault(t, []).append(fn)

            for k in range(T):
                at(k + 4 if k + 4 < T else -1,
                   (lambda k=k: mini_dense_tile(1, k, HID, wcat1)))
            start_of = {0: 18, 1: 30, 2: 42, 3: -1, 4: -1}
            counts = {}
            leftovers = []
            for (j, gt0, nb, src_ap) in l1_dense_blocks():
                s = start_of[j]
                slot = -1
                if s >= 0:
                    k = counts.get(j, 0)
                    counts[j] = k + 1
                    slot = s + k // 2
                if slot < 0 or slot >= T:
                    leftovers.append((gt0 // T >= 4, j, gt0, nb, src_ap))
                    continue
                at(slot, (lambda w=wcat1, g=gt0, n=nb, a=src_ap:
                          dense_block(1, w, g, n, a)))
            # post-loop leftovers: half-0 bands first so layer-1 lo-gathers
            # unblock as early as possible
            for (_, j, gt0, nb, src_ap) in sorted(
                leftovers, key=lambda x: (x[0], x[1])
            ):
                at(-1, (lambda w=wcat1, g=gt0, n=nb, a=src_ap:
                        dense_block(1, w, g, n, a)))

            layer(0, interleave=sched)
            layer(1)

    nc.compile()
    return nc


# ---------------------------------------------------------------------------
# Host entry point
# ---------------------------------------------------------------------------
def kernel(feats, src, dst, W0, al0, ar0, resw0, resb0, g0, b0,
           W1, al1, ar1, resw1, resb1, g1, b1, predw, predb):
    f32 = np.float32
    feats = np.asarray(feats, f32)
    W0 = np.asarray(W0, f32)
    al0 = np.asarray(al0, f32)
    ar0 = np.asarray(ar0, f32)
    resw0 = np.asarray(resw0, f32)
    W1 = np.asarray(W1, f32)
    al1 = np.asarray(al1, f32)
    ar1 = np.asarray(ar1, f32)
    resw1 = np.asarray(resw1, f32)
    predw_np = np.asarray(predw, f32)

    ep = _prep_edges(src, dst)
    nc = _build_program(ep)

    feats_pad = np.zeros((N_PAD, IN), f32)
    feats_pad[:N] = feats

    def fold(W, a):
        return (W.reshape(W.shape[0], H, HID) * a[None]).sum(-1)

    # wcat: [W (256) | el (4) | er (4) | res (64)]
    wcat0 = np.concatenate([W0, fold(W0, al0), fold(W0, ar0), resw0], axis=1)
    wcat1 = np.concatenate([W1, fold(W1, al1), fold(W1, ar1), resw1], axis=1)

    aux = np.zeros((P, 8 * 64), f32)
    aux[:, 0:64] = np.asarray(g0, f32)[None]
    aux[:, 64:128] = np.asarray(b0, f32)[None]
    aux[:, 128:192] = np.asarray(g1, f32)[None]
    aux[:, 192:256] = np.asarray(b1, f32)[None]
    aux[:, 256:320] = np.asarray(resb0, f32)[None]
    aux[:, 320:384] = np.asarray(resb1, f32)[None]
    aux[:, 384:448] = np.asarray(predb, f32)[None]
    aux[:, 448] = EPS

    ident = np.eye(P, dtype=f32)

    featsT = np.ascontiguousarray(feats_pad.T.astype(ml_dtypes.bfloat16))

    shared = {
        "featsT": featsT,
        "wcat0": np.ascontiguousarray(wcat0.astype(ml_dtypes.bfloat16)),
        "wcat1": np.ascontiguousarray(wcat1.astype(ml_dtypes.bfloat16)),
        "predw": predw_np,
        "aux": aux,
        "ident": ident,
    }
    in_maps = [
        {
            **shared,
            "featsT_own": np.ascontiguousarray(
                featsT[:, c * NPC : (c + 1) * NPC]
            ),
            "idx16": ep["idx16"][c],
            "indhot": ep["ind_np"][c],
            "indhotT": ep["indT_np"][c],
        }
        for c in range(NCORES)
    ]

    trace = os.environ.get("GAT_TRACE", "0") == "1"
    res = run_bass_kernel_spmd(
        nc, in_maps, core_ids=list(range(NCORES)), trace=trace
    )
    if trace and res.exec_time_ns is not None:
        print(f"HW exec time: {res.exec_time_ns} ns")
        if res.instructions_and_trace is not None:
            print(f"trace: {res.instructions_and_trace[1]}")

    out = np.concatenate([res.results[c]["out"] for c in range(NCORES)], axis=0)
    return np.ascontiguousarray(out[:N]).astype(np.float32)
